# revision 34
# baseline (speedup 1.0000x reference)
"""Trainium2 Bass kernel for nn_MHC (dense transformer block: QKV -> causal
attention -> conv1d(k=3) -> causal attention (same K/V) -> out proj).

Sharding over 8 NeuronCores: data-parallel on batch (2) x tensor-parallel on
heads (16 heads -> 4 per core). Cores 0-3 own batch 0, cores 4-7 batch 1.
Chunked AllGathers (one per 512-token block, per batch group of 4 cores)
exchange attention-1 context so each core can run the channel-mixing conv for
its own output channels; the chunks overlap with attention compute.

Everything on-chip lives in transposed [channel, token] layout in bf16; the
host pre-transposes x and all weights so the device never transposes anything.
The final out-proj is computed as per-core bf16 partial sums over local
channels; the host adds the 4 partials per batch.
"""

import numpy as np
import ml_dtypes

import concourse.bacc as bacc
import concourse.mybir as mybir
import concourse.tile as tile
from concourse.bass import ts, AP
from concourse.bass_utils import run_bass_kernel_spmd

# Problem shapes (hardcoded per contract)
B, S, D = 2, 2048, 1024
H, DH = 16, 64
N_CORES = 8
HPC = 4          # heads per core
CL = HPC * DH    # 256 local channels
KT = D // 128    # 8 k-tiles over the model dim
NJ = S // 512    # 4 t-blocks of 512
NS = S // 128    # 16 s-tiles of 128
GROUPS = [[0, 1, 2, 3], [4, 5, 6, 7]]
WORLD = [[0, 1, 2, 3, 4, 5, 6, 7]]

F32 = mybir.dt.float32
BF16 = mybir.dt.bfloat16
EXP = mybir.ActivationFunctionType.Exp
COPYF = mybir.ActivationFunctionType.Copy
MULT = mybir.AluOpType.mult
ADD = mybir.AluOpType.add
DIV = mybir.AluOpType.divide

_CACHE = {}


def build_kernel(collective=True, taps=False):
    key = ("nc", collective, taps)
    if key in _CACHE:
        return _CACHE[key]
    nc = bacc.Bacc("TRN2", target_bir_lowering=False, debug=False,
                   num_devices=N_CORES if collective else 1)

    # ---- I/O ----
    xT_d = nc.dram_tensor("xT", [D, S], BF16, kind="ExternalInput")
    wqk_d = nc.dram_tensor("wqk", [D, 4 * 128], BF16, kind="ExternalInput")
    wv_d = nc.dram_tensor("wv", [D, CL], BF16, kind="ExternalInput")
    qkb_d = nc.dram_tensor("qkb", [4, 128], F32, kind="ExternalInput")
    vbb_d = nc.dram_tensor("vbb", [128, CL], F32, kind="ExternalInput")
    cw_d = nc.dram_tensor("cw", [3, D, CL], BF16, kind="ExternalInput")
    cb_d = nc.dram_tensor("cb", [2, 128], F32, kind="ExternalInput")
    ow_d = nc.dram_tensor("ow", [CL, D], BF16, kind="ExternalInput")
    tri_d = nc.dram_tensor("tri", [128, 128], BF16, kind="ExternalInput")
    outT_d = nc.dram_tensor("outT", [D, S], BF16, kind="ExternalOutput")
    if taps:
        dctx1_d = nc.dram_tensor("dctx1", [128, 2, S], BF16, kind="ExternalOutput")
        dctxg_d = nc.dram_tensor("dctxg", [128, KT, 514], BF16, kind="ExternalOutput")
        dq2_d = nc.dram_tensor("dq2", [128, 2, S], BF16, kind="ExternalOutput")
        dctx2_d = nc.dram_tensor("dctx2", [128, 2, S], BF16, kind="ExternalOutput")

    xT_v = xT_d.ap().rearrange("(kt p) t -> p kt t", p=128)
    outT_v = outT_d.ap().rearrange("(m p) t -> p m t", p=128)

    with tile.TileContext(nc) as tc:
        with (
            tc.tile_pool(name="w", bufs=1) as wp,
            tc.tile_pool(name="big", bufs=1) as bigp,
            tc.tile_pool(name="xs", bufs=4) as xsp,
            tc.tile_pool(name="p", bufs=3) as pp,
            tc.tile_pool(name="bcs", bufs=2) as bcsp,
            tc.tile_pool(name="rc", bufs=2) as rcp,
            tc.tile_pool(name="ps", bufs=1, space="PSUM") as psp,
            tc.tile_pool(name="dram", bufs=1, space="DRAM") as dramp,
        ):
            # ---- warm-up barrier collective over all 8 cores ----
            # Pure sync: warms the ncfw/CC path before the first real
            # AllGather. Nothing reads the (garbage) payload.
            if collective:
                bar_in = dramp.tile([16, 64], BF16)
                bar_out = dramp.tile([128, 64], BF16)
                nc.gpsimd.collective_compute(
                    "AllGather", mybir.AluOpType.bypass, replica_groups=WORLD,
                    ins=[bar_in.opt()], outs=[bar_out.opt()])

            with nc.named_scope("main"):
                # ---- load weights / constants (phase-A-critical first) ----
                # wqk split per k-tile so the first matmul (kt=0) starts
                # after 1/8th of the load
                wqk = wp.tile([128, KT, 512], BF16)
                wqk_v = wqk_d.ap().rearrange("(kt p) m -> p kt m", p=128)
                for kt in range(KT):
                    nc.sync.dma_start(wqk[:, kt, :], wqk_v[:, kt, :])
                qkb = wp.tile([128, 4], F32)
                nc.sync.dma_start(qkb[:], qkb_d.ap().rearrange("m p -> p m"))
                wv = wp.tile([128, KT, CL], BF16)
                nc.sync.dma_start(wv[:], wv_d.ap().rearrange("(kt p) c -> p kt c", p=128))
                vbb = wp.tile([128, CL], F32)
                nc.sync.dma_start(vbb[:], vbb_d.ap())
                tri = wp.tile([128, 128], BF16)
                nc.sync.dma_start(tri[:], tri_d.ap())

                # ---- persistent activations ----
                # q/k pair tiles: rows 0-63 head (2*kp), 64-127 head (2*kp+1)
                qpair = [bigp.tile([128, S], BF16, tag=f"qp{k}", name=f"qp{k}") for k in range(2)]
                kpair = [bigp.tile([128, S], BF16, tag=f"kp{k}", name=f"kp{k}") for k in range(2)]
                # v in token-major: [s-part, s_tile, local head, 64 v | 1]
                v_sb = bigp.tile([128, NS, HPC, 65], BF16)
                nc.vector.memset(v_sb[:, :, :, 64:65], 1.0)
                ctx1 = bigp.tile([128, 2, S], BF16)   # local attn-1 context
                ctx2 = bigp.tile([128, 2, S], BF16)   # local attn-2 context
                # gathered ctx: one tile per t-block with 1-token halo on each
                # side, so conv(j) depends only on AG chunks j-1..j+1
                ctxg = [bigp.tile([128, KT, 514], BF16, tag=f"cg{j}", name=f"cg{j}")
                        for j in range(NJ)]
                nc.vector.memset(ctxg[0][:, :, 0:1], 0.0)
                nc.vector.memset(ctxg[NJ - 1][:, :, 513:514], 0.0)

                cc_in = [dramp.tile([CL, 512], BF16, name=f"cin{j}") for j in range(NJ)]
                cc_out = [dramp.tile([D, 512], BF16, name=f"cout{j}") for j in range(NJ)]
                # DRAM bounce buffers for the softmax-reciprocal partition
                # broadcast (stride-0 DRAM reads replicate across partitions)
                rcd = [dramp.tile([1, 512], F32, name=f"rcd{i}") for i in range(4)]

                # x prefetch: all 4 t-blocks up front so later epilogue DMA
                # triggers never delay the input loads on the SP queue
                xts = []
                for j in range(NJ):
                    xt = xsp.tile([128, KT, 512], BF16, tag="xt")
                    if j == 0:  # per-kt split: first qk matmul starts sooner
                        for kt in range(KT):
                            nc.sync.dma_start(xt[:, kt, :], xT_v[:, kt, ts(j, 512)])
                    else:
                        nc.sync.dma_start(xt[:], xT_v[:, :, ts(j, 512)])
                    xts.append(xt)

                # ---------------- phase A block: QKV for t-block j ------------
                def qkv_block(j):
                    xt = xts[j]
                    # q (m=0,1) / k (m=2,3) transposed: [channels, t]
                    for m in range(4):
                        ps = psp.tile([128, 512], F32, tag="mm", bufs=2)
                        for kt in range(KT):
                            nc.tensor.matmul(ps[:], wqk[:, kt, ts(m, 128)],
                                             xt[:, kt, :],
                                             start=(kt == 0), stop=(kt == KT - 1))
                        dst = qpair[m] if m < 2 else kpair[m - 2]
                        nc.vector.tensor_scalar(dst[:, ts(j, 512)], ps[:],
                                                qkb[:, m:m + 1], None, ADD)
                    # v token-major: [t, c] for the 4 s-tiles of this block
                    for u in range(4):
                        ps = psp.tile([128, CL], F32, tag="mm", bufs=2)
                        for kt in range(KT):
                            nc.tensor.matmul(ps[:], xt[:, kt, ts(u, 128)],
                                             wv[:, kt, :],
                                             start=(kt == 0), stop=(kt == KT - 1))
                        st_i = 4 * j + u
                        nc.vector.tensor_tensor(
                            v_sb[:, st_i, :, 0:64],
                            ps.rearrange("p (h e) -> p h e", e=64),
                            vbb.rearrange("p (h e) -> p h e", e=64), ADD)

                # ---------------- attention block (kp, j) ---------------------
                def attn_block(dst, kp, j):
                    cps = [psp.tile([65, 512], F32, tag="ctx", name="ctx", bufs=2)
                           for _ in range(2)]
                    i_last = 4 * j + 3
                    for i in range(4 * j + 4):
                        r = i - 4 * j
                        c0 = 128 * r if r > 0 else 0
                        st = psp.tile([128, 2, 512], F32, tag="st", bufs=2)
                        for hh in range(2):
                            row = slice(64 * hh, 64 * hh + 64)
                            nc.tensor.matmul(st[:, hh, c0:512],
                                             kpair[kp][row, ts(i, 128)],
                                             qpair[kp][row, j * 512 + c0:(j + 1) * 512])
                        p = pp.tile([128, 2, 512], BF16, tag="p")
                        nc.scalar.activation(p[:, :, c0:512], st[:, :, c0:512], EXP)
                        if r >= 0:
                            # gpsimd, not vector: keeps the PE-feeding mask op
                            # out of the DVE queue behind softmax reciprocals
                            for hh in range(2):
                                nc.gpsimd.tensor_tensor(
                                    p[:, hh, c0:c0 + 128], p[:, hh, c0:c0 + 128],
                                    tri[:], MULT)
                        for hh in range(2):
                            nc.tensor.matmul(
                                cps[hh][:, c0:512],
                                v_sb[:, i, 2 * kp + hh, :], p[:, hh, c0:512],
                                start=(i == 0), stop=(i == i_last))
                    for hh in range(2):
                        cp = cps[hh]
                        # one fast ACT copy frees the PSUM ctx bank for the
                        # next block; the slow reciprocal + broadcast +
                        # normalize then run entirely off the PE/PSUM path
                        sct = bcsp.tile([65, 512], F32, tag="sc")
                        nc.scalar.activation(sct[:], cp[:], COPYF)
                        rc = rcp.tile([1, 512], F32, tag="rc")
                        with nc.allow_low_precision(reason="softmax denom recip"):
                            nc.vector.reciprocal(rc[:], sct[64:65, :])
                        rd = rcd[2 * (j % 2) + hh]
                        nc.sync.dma_start(rd.opt()[:, :], rc[:])
                        src = rd.opt()[:, :]
                        bc = bcsp.tile([64, 512], F32, tag="bc")
                        nc.sync.dma_start(
                            bc[:], AP(src.tensor, src.offset, [[0, 64], [1, 512]]))
                        nc.vector.tensor_tensor(
                            dst[64 * hh:64 * hh + 64, kp, ts(j, 512)],
                            sct[0:64, :], bc[:], MULT)

                # ============ phase A + attention 1, interleaved per j ========
                for j in range(NJ):
                    qkv_block(j)
                    for kp in range(2):
                        attn_block(ctx1, kp, j)
                    # chunked AllGather of this t-block's attn-1 context
                    nc.sync.dma_start(
                        cc_in[j].opt().rearrange("(kt p) t -> p kt t", p=128),
                        ctx1[:, :, ts(j, 512)])
                    if collective:
                        nc.gpsimd.collective_compute(
                            "AllGather", mybir.AluOpType.bypass,
                            replica_groups=GROUPS,
                            ins=[cc_in[j].opt()], outs=[cc_out[j].opt()])
                    else:
                        for g4 in range(4):
                            nc.sync.dma_start(
                                cc_out[j].opt()[CL * g4:CL * (g4 + 1), :],
                                cc_in[j].opt()[:])

                # gathered-ctx unpack DMAs emitted AFTER the whole attn-1
                # loop: their triggers wait on the AllGathers, and emitting
                # them inline would head-of-line-block the SP DMA queue that
                # the softmax-epilogue DMAs (which feed the PE) run on
                for j in range(NJ):
                    co = cc_out[j].opt().rearrange("(kt p) t -> p kt t", p=128)
                    nc.sync.dma_start(ctxg[j][:, :, 1:513], co)
                    if j > 0:      # first token -> right halo of block j-1
                        nc.sync.dma_start(ctxg[j - 1][:, :, 513:514], co[:, :, 0:1])
                    if j < NJ - 1:  # last token -> left halo of block j+1
                        nc.sync.dma_start(ctxg[j + 1][:, :, 0:1], co[:, :, 511:512])

                # ---- late-needed weights (DMA hidden behind attention 1) ----
                cw = wp.tile([128, 3, KT, CL], BF16)
                nc.sync.dma_start(cw[:], cw_d.ap().rearrange("a (kt p) o -> p a kt o", p=128))
                cb = wp.tile([128, 2], F32)
                nc.sync.dma_start(cb[:], cb_d.ap().rearrange("m p -> p m"))
                ow = wp.tile([128, 2, 8, 128], BF16)
                nc.sync.dma_start(
                    ow[:], ow_d.ap().rearrange("(kt p) (m q) -> p kt m q", p=128, q=128))

                # ============ conv -> attention 2 -> out proj, per j ==========
                # conv1d (k=3): conv_out[o,t] = sum_{tap,i} cw[tap][i,o] *
                #               ctx[i, t+tap-1]   (ctxg[j] is t-block j with a
                #               1-token halo on each side)
                def conv_block(j):
                    for ot in range(2):
                        ps = psp.tile([128, 512], F32, tag="mm", bufs=2)
                        first = True
                        for kt in range(KT):
                            for tap in range(3):
                                nc.tensor.matmul(
                                    ps[:], cw[:, tap, kt, ts(ot, 128)],
                                    ctxg[j][:, kt, tap: tap + 512],
                                    start=first,
                                    stop=(kt == KT - 1 and tap == 2))
                                first = False
                        nc.vector.tensor_scalar(qpair[ot][:, ts(j, 512)], ps[:],
                                                cb[:, ot:ot + 1], None, ADD)

                def out_block(j):
                    for m in range(8):
                        ps = psp.tile([128, 512], F32, tag="mm", bufs=2)
                        for kt in range(2):
                            nc.tensor.matmul(ps[:], ow[:, kt, m, :],
                                             ctx2[:, kt, ts(j, 512)],
                                             start=(kt == 0), stop=(kt == 1))
                        ob = bcsp.tile([128, 512], BF16, tag="ob", bufs=3)
                        nc.vector.tensor_copy(out=ob[:], in_=ps[:])
                        nc.sync.dma_start(outT_v[:, m, ts(j, 512)], ob[:])

                # software pipeline: conv(j+1) fills the PE while attn2(j)'s
                # softmax epilogue drains; out(j) trails attn2(j+1) so it
                # never waits on a fresh epilogue
                conv_block(0)
                for j in range(NJ):
                    for kp in range(2):
                        attn_block(ctx2, kp, j)
                    if j + 1 < NJ:
                        conv_block(j + 1)
                    if j >= 1:
                        out_block(j - 1)
                out_block(NJ - 1)

                if taps:
                    nc.sync.dma_start(dctx1_d.ap(), ctx1[:])
                    nc.sync.dma_start(dctxg_d.ap(), ctxg[1][:])
                    for ot in range(2):
                        nc.sync.dma_start(dq2_d.ap()[:, ot, :], qpair[ot][:])
                    nc.sync.dma_start(dctx2_d.ap(), ctx2[:])

    nc.compile()
    _CACHE[key] = nc
    return nc


def prep_inputs(x, Wqkv_w, Wqkv_b, conv_w, conv_b, out_w, out_b):
    """Build the 8 per-core input maps from the full problem inputs."""
    bf16 = ml_dtypes.bfloat16
    x = np.asarray(x, np.float32)
    Wqkv_w = np.asarray(Wqkv_w, np.float32)
    Wqkv_b = np.asarray(Wqkv_b, np.float32)
    conv_w = np.asarray(conv_w, np.float32)
    conv_b = np.asarray(conv_b, np.float32)
    out_w = np.asarray(out_w, np.float32)

    scale = 1.0 / np.sqrt(DH).astype(np.float32)
    tri = (np.arange(128)[None, :] >= np.arange(128)[:, None]).astype(bf16)

    in_maps = []
    for g in range(N_CORES):
        b, hg = g // 4, g % 4
        h0 = HPC * hg
        # q/k row blocks, m-tiles: [q pair0, q pair1, k pair0, k pair1]
        rows = []
        biases = []
        for blk, sc in ((0, scale), (1, 1.0)):
            for pr in range(2):
                r0 = blk * D + (h0 + 2 * pr) * DH
                rows.append(Wqkv_w[r0:r0 + 128, :] * sc)
                biases.append(Wqkv_b[r0:r0 + 128] * sc)
        wqk = np.ascontiguousarray(np.concatenate(rows, axis=0).T).astype(bf16)
        qkb = np.stack(biases, axis=0)  # [4, 128]
        c0 = CL * hg
        wv = np.ascontiguousarray(
            Wqkv_w[2 * D + c0:2 * D + c0 + CL, :].T).astype(bf16)
        vbb = np.ascontiguousarray(
            np.broadcast_to(Wqkv_b[2 * D + c0:2 * D + c0 + CL], (128, CL)))
        cw = np.ascontiguousarray(
            (conv_w[c0:c0 + CL, :, :] * scale).transpose(2, 1, 0)
        ).astype(bf16)  # [3, D, CL]
        cb = (conv_b[c0:c0 + CL] * scale).reshape(2, 128).astype(np.float32)
        owm = np.ascontiguousarray(out_w[:, c0:c0 + CL].T).astype(bf16)
        in_maps.append({
            "xT": np.ascontiguousarray(x[b].T).astype(bf16),
            "wqk": wqk, "wv": wv,
            "qkb": np.ascontiguousarray(qkb),
            "vbb": vbb, "cw": cw,
            "cb": np.ascontiguousarray(cb),
            "ow": owm, "tri": np.ascontiguousarray(tri),
        })
    return in_maps


def postprocess(results, out_b):
    out_b = np.asarray(out_b, np.float32)
    out = np.empty((B, S, D), np.float32)
    for b in range(B):
        acc = np.zeros((D, S), np.float32)
        for g in GROUPS[b]:
            acc += np.asarray(results[g]["outT"], np.float32)
        out[b] = acc.T + out_b[None, :]
    return out


def kernel(x, Wqkv_w, Wqkv_b, conv_w, conv_b, out_w, out_b):
    nc = build_kernel()
    in_maps = prep_inputs(x, Wqkv_w, Wqkv_b, conv_w, conv_b, out_w, out_b)
    res = run_bass_kernel_spmd(nc, in_maps, core_ids=list(range(N_CORES)))
    return postprocess(res.results, out_b)
